# revision 3
# baseline (speedup 1.0000x reference)
"""Trainium2 Bass kernel for DifferentiableEdgeWeighting (8-core SPMD).

Sharding: edges partitioned by source node; core c owns sources
[c*6250, (c+1)*6250). Within a core, segments (source nodes) are bin-packed
onto 128 lanes; each lane stores its segments' edges contiguously (runs
padded to multiples of 4) so the scatter-softmax becomes segmented scans
along the free dimension.

dma_gather cost is pure Q7 descriptor generation (~8 ns/idx, independent of
bytes), so the es side gathers ONE 512B descriptor per 4 slots from a
quad-duplicated table (row r = [es_r,es_r,es_r,es_r]); ea stays per-slot
from the pair-packed table (parity-selected on DVE). catT for the MLP is
built by HWDGE xbar dma transposes (SP/Act queues) instead of PE transposes.

Per-edge pipeline (2048-edge batches of 16 PE tiles):
  - norm: DVE sub/mul + free-dim reduce (bf16).
  - MLP: h = cat @ W1' with W1' = W1*diag(|W2|), columns sign-sorted; gate
    logit = sum(relu(h)[:, :P]) - sum(relu(h)[:, P:]) via free-dim reduces.
  - softmax: exp -> segmented prefix-scan -> totals broadcast back through a
    reversed scan (free-dim flips via PE identity/anti-identity transposes).
"""

import sys
import numpy as np

sys.path.insert(0, "/opt/trn_rl_repo")

NUM_S = 50000
NUM_A = 50000
D = 64
H = 128
E = 1_000_000
NCORES = 8
S_PER_CORE = NUM_S // NCORES  # 6250
LANES = 128
W = 1152                      # slots per lane (quad-padded runs; mult of 128)
TILE = 128
TPB = 16                      # tiles per batch
BATCH = TILE * TPB            # 2048
NQB = BATCH // 4              # es quad idxs per batch (512)
NSLOT = LANES * W

_compiled = {}


def _lane_pack(sizes):
    """Greedy LPT bin-packing of segments onto LANES lanes by padded size."""
    import heapq
    lane_of_seg = np.zeros(sizes.shape[0], dtype=np.int32)
    order = np.argsort(-sizes, kind="stable")
    heap = [(0, l) for l in range(LANES)]
    heapq.heapify(heap)
    for s in order:
        c = int(sizes[s])
        if c == 0:
            continue
        load, l = heapq.heappop(heap)
        lane_of_seg[s] = l
        heapq.heappush(heap, (load + c, l))
    return lane_of_seg


def _wrap16(v):
    """dma_gather index layout: index i lives at [i % 16, i // 16],
    replicated across the 8 Q7 cores (partition groups of 16)."""
    w16 = v.reshape(-1, 16).T  # [16, n/16]
    return np.ascontiguousarray(np.tile(w16, (8, 1)).astype(np.int16))


def _prep_core(c, s_idx, a_idx):
    ids = np.nonzero(s_idx // S_PER_CORE == c)[0]
    sl = (s_idx[ids] - c * S_PER_CORE).astype(np.int32)
    av = a_idx[ids].astype(np.int32)

    o = np.argsort(sl, kind="stable")
    ids, sl, av = ids[o], sl[o], av[o]

    counts = np.bincount(sl, minlength=S_PER_CORE)
    pad4 = (counts + 3) // 4 * 4
    lane_of_seg = _lane_pack(pad4)

    # per-lane segment lists (ascending seg id), then slot streams
    lane_key = lane_of_seg[sl]
    seg_start = np.zeros(S_PER_CORE + 1, dtype=np.int64)
    np.cumsum(counts, out=seg_start[1:])

    grp = np.full((LANES, W), -1, dtype=np.int32)
    orig = np.full((LANES, W), -1, dtype=np.int64)
    amat = np.zeros((LANES, W), dtype=np.int32)
    pos = np.zeros(LANES, dtype=np.int64)
    segs_by_lane = [[] for _ in range(LANES)]
    for s in np.nonzero(counts)[0]:
        segs_by_lane[lane_of_seg[s]].append(s)
    for l in range(LANES):
        p = 0
        for s in segs_by_lane[l]:
            cnt = int(counts[s])
            pc = int(pad4[s])
            e0 = seg_start[s]
            if p + pc > W:
                raise RuntimeError(f"lane overflow: {p + pc} > {W}")
            grp[l, p:p + pc] = s
            orig[l, p:p + cnt] = ids[e0:e0 + cnt]
            amat[l, p:p + cnt] = av[e0:e0 + cnt]
            p += pc
        pos[l] = p
    max_load = int(pos.max())

    valid = orig >= 0
    same = np.zeros((LANES, W), dtype=bool)
    same[:, 1:] = grp[:, 1:] == grp[:, :-1]
    cont = same.astype(np.float32)
    is_end = np.ones((LANES, W), dtype=bool)
    is_end[:, :-1] = ~same[:, 1:]
    endm = is_end.astype(np.float32)
    rev_cont = np.ascontiguousarray((1.0 - endm)[:, ::-1])
    validf = valid.astype(np.float32)
    invalf = (~valid).astype(np.float32)

    # slot stream order: slot t = (lane t%128, pos t//128) -> column-major
    a_slots = np.ascontiguousarray(amat.T).reshape(-1)
    qgrid = grp[:, 0::4]                       # [LANES, W//4]
    q_slots = np.ascontiguousarray(qgrid.T).reshape(-1)
    q_slots = np.where(q_slots < 0, 0, q_slots)
    parm = (amat & 1).astype(np.uint8)

    return {
        "orig": orig, "valid": valid, "max_load": max_load,
        "cont": cont, "rev_cont": rev_cont, "endm": endm,
        "validf": validf, "invalf": invalf,
        "idx_s16": _wrap16(q_slots),
        "idx_a16": _wrap16(a_slots >> 1),
        "parm": parm,
    }


def _split_excess_waits(nc, mybir):
    """Several ISA structs (SWDGE DMA, Ldweights, DVE copies, NoOp) carry a
    single sync-wait slot, but Tile can emit 2+ waits on one instruction.
    Move excess waits onto NoOps inserted just before the instruction on the
    same engine queue (the sequencer stalls on each in order)."""
    fixn = [0]

    def fix_block(blk):
        new_list = []
        for inst in blk.instructions:
            si = inst.sync_info
            if si is not None and si.on_wait and len(si.on_wait) > 1:
                for w in list(si.on_wait[:-1]):
                    nop = mybir.InstNoOp(
                        name=f"I-waitfix-{fixn[0]}",
                        ins=[],
                        outs=[],
                    )
                    fixn[0] += 1
                    nop.engine = inst.engine
                    nop.sync_info = mybir.SyncInfo(on_wait=[w], on_update=[])
                    try:
                        nc.register_instruction(nop, overwrite=True)
                    except Exception:
                        pass
                    new_list.append(nop)
                si.on_wait = [si.on_wait[-1]]
            new_list.append(inst)
        blk.instructions[:] = new_list

    def walk(blocks):
        for b in blocks:
            fix_block(b)
            inner = getattr(b, "blocks", None)
            if inner:
                walk(inner)

    for f in nc.m.functions:
        walk(f.blocks)


def _build_program(P, NB):
    from concourse import bass, mybir, library_config
    from concourse.bacc import Bacc
    import concourse.tile as tile
    from concourse import tile_sem_assignment as _tsa
    _tsa.NUM_SWDGE_GLOBAL_SEMS = 1
    _tsa.NUM_HWDGE_SEMS = 1

    f32 = mybir.dt.float32
    bf16 = mybir.dt.bfloat16
    i16 = mybir.dt.int16
    Alu = mybir.AluOpType
    Act = mybir.ActivationFunctionType

    nc = Bacc()

    NWA = LANES * W // 16        # ea idx columns (full grid)
    NWS = LANES * (W // 4) // 16  # es quad idx columns
    tab_s = nc.declare_dram_parameter("tab_s", [S_PER_CORE, 256], bf16, isOutput=False)
    tab_a = nc.declare_dram_parameter("tab_a", [NUM_A // 2, 128], bf16, isOutput=False)
    idx_s_d = nc.declare_dram_parameter("idx_s", [LANES, NWS], i16, isOutput=False)
    idx_a_d = nc.declare_dram_parameter("idx_a", [LANES, NWA], i16, isOutput=False)
    par_d = nc.declare_dram_parameter("parm", [LANES, W], mybir.dt.uint8, isOutput=False)
    cont_d = nc.declare_dram_parameter("cont", [LANES, W], f32, isOutput=False)
    rcont_d = nc.declare_dram_parameter("rev_cont", [LANES, W], f32, isOutput=False)
    endm_d = nc.declare_dram_parameter("endm", [LANES, W], f32, isOutput=False)
    validm_d = nc.declare_dram_parameter("validm", [LANES, W], f32, isOutput=False)
    invalm_d = nc.declare_dram_parameter("invalm", [LANES, W], f32, isOutput=False)
    w1p_d = nc.declare_dram_parameter("w1p", [H, H], f32, isOutput=False)
    b1row_d = nc.declare_dram_parameter("b1row", [1, BATCH], f32, isOutput=False)
    scal_d = nc.declare_dram_parameter("scal", [LANES, 2], f32, isOutput=False)
    ident_d = nc.declare_dram_parameter("identm", [128, 128], f32, isOutput=False)
    antid_d = nc.declare_dram_parameter("antidm", [128, 128], f32, isOutput=False)

    wout_d = nc.declare_dram_parameter("wout", [LANES, W], f32, isOutput=True)
    cout_d = nc.declare_dram_parameter("cout", [LANES, W], f32, isOutput=True)

    with tile.TileContext(nc) as tc:
        with (
            tc.tile_pool(name="persist", bufs=1) as pp,
            tc.tile_pool(name="work", bufs=2) as wp,
            tc.tile_pool(name="psA", bufs=2, space="PSUM") as psA,
            tc.tile_pool(name="psB", bufs=1, space="PSUM") as psB,
        ):
            nc.gpsimd.load_library(library_config.mlp)

            idx_s = pp.tile([LANES, NWS], i16, tag="idx_s")
            idx_a = pp.tile([LANES, NWA], i16, tag="idx_a")
            parm = pp.tile([LANES, W], mybir.dt.uint8, tag="parm")
            contm = pp.tile([LANES, W], f32, tag="contm")
            rcontm = pp.tile([LANES, W], f32, tag="rcontm")
            endmm = pp.tile([LANES, W], f32, tag="endmm")
            validm = pp.tile([LANES, W], f32, tag="validm")
            invalm = pp.tile([LANES, W], f32, tag="invalm")
            nsq = pp.tile([LANES, W], f32, tag="nsq")
            gpos = pp.tile([LANES, W], f32, tag="gpos")
            gneg = pp.tile([LANES, W], f32, tag="gneg")
            costp = pp.tile([LANES, W], f32, tag="costp")
            exv = pp.tile([LANES, W], f32, tag="exv")
            tmp1 = pp.tile([LANES, W], f32, tag="tmp1")
            tmp2 = pp.tile([LANES, W], f32, tag="tmp2")
            w1p = pp.tile([H, H], bf16, tag="w1p")
            b1row = pp.tile([1, BATCH], bf16, tag="b1row")
            onesc = pp.tile([1, H], bf16, tag="onesc")
            scal = pp.tile([LANES, 2], f32, tag="scal")
            ident_f = pp.tile([128, 128], f32, tag="ident_f")
            antid_f = pp.tile([128, 128], f32, tag="antid_f")

            nc.sync.dma_start(out=idx_s[:, :], in_=idx_s_d[:, :])
            nc.sync.dma_start(out=idx_a[:, :], in_=idx_a_d[:, :])
            nc.sync.dma_start(out=parm[:, :], in_=par_d[:, :])
            nc.sync.dma_start(out=contm[:, :], in_=cont_d[:, :])
            nc.sync.dma_start(out=rcontm[:, :], in_=rcont_d[:, :])
            nc.sync.dma_start(out=endmm[:, :], in_=endm_d[:, :])
            nc.sync.dma_start(out=validm[:, :], in_=validm_d[:, :])
            nc.sync.dma_start(out=invalm[:, :], in_=invalm_d[:, :])
            nc.sync.dma_start(out=scal[:, :], in_=scal_d[:, :])
            nc.sync.dma_start(out=ident_f[:, :], in_=ident_d[:, :])
            nc.sync.dma_start(out=antid_f[:, :], in_=antid_d[:, :])
            nc.gpsimd.dma_start(out=w1p[:, :], in_=w1p_d[:, :])       # f32->bf16
            nc.gpsimd.dma_start(out=b1row[:, :], in_=b1row_d[:, :])   # f32->bf16
            nc.vector.memset(onesc[:, :], 1.0)
            # batches only cover the real prefix; zero the tails read by the
            # packed phase
            nc.vector.memset(nsq[:, :], 0.0)
            nc.vector.memset(gpos[:, :], 0.0)
            nc.vector.memset(gneg[:, :], 0.0)

            for b in range(NB):
                c0 = b * TPB
                ia0 = b * (BATCH // 16)   # ea wrapped-idx column base
                is0 = b * (NQB // 16)     # es wrapped-idx column base
                es_q = wp.tile([128, 4, 256], bf16, tag="es_q")
                ea_t = wp.tile([128, TPB, 128], bf16, tag="ea_t")
                nc.gpsimd.dma_gather(
                    out_ap=es_q[:, :, :], in_ap=tab_s[:, :],
                    idxs_ap=idx_s[:, is0:is0 + NQB // 16],
                    num_idxs=NQB, num_idxs_reg=NQB, elem_size=256,
                    single_packet=False)
                nc.gpsimd.dma_gather(
                    out_ap=ea_t[:, :, :], in_ap=tab_a[:, :],
                    idxs_ap=idx_a[:, ia0:ia0 + BATCH // 16],
                    num_idxs=BATCH, num_idxs_reg=BATCH, elem_size=128,
                    single_packet=False)

                es16 = es_q[:, :, :].rearrange("p a (b c) -> p (a b) c", b=4, c=64)

                # comb[:, t, :] = [es(t) | easel(t)]; parity select writes the
                # ea half in place (easel = par ? ea_odd : ea_even)
                comb = wp.tile([128, TPB, 128], bf16, tag="comb")
                par_bc = parm[:, c0:c0 + TPB].to_broadcast([128, TPB, 64])
                nc.vector.tensor_copy(comb[:, :, 0:64], es16)
                nc.vector.tensor_copy(comb[:, :, 64:128], ea_t[:, :, 0:64])
                nc.vector.copy_predicated(comb[:, :, 64:128], par_bc,
                                          ea_t[:, :, 64:128])

                diff = wp.tile([128, TPB, 64], bf16, tag="diff")
                nc.vector.tensor_tensor(
                    out=diff[:, :, :], in0=comb[:, :, 0:64],
                    in1=comb[:, :, 64:128], op=Alu.subtract)
                dsq = wp.tile([128, TPB, 64], bf16, tag="dsq")
                nc.vector.tensor_tensor(
                    out=dsq[:, :, :], in0=diff[:, :, :], in1=diff[:, :, :],
                    op=Alu.mult)
                nc.vector.tensor_reduce(
                    out=nsq[:, c0:c0 + TPB], in_=dsq[:, :, :],
                    axis=mybir.AxisListType.X, op=Alu.add)

                # catT via HWDGE xbar transposes (SP + Act queues)
                catT = wp.tile([128, TPB, 128], bf16, tag="catT")
                for i in range(TPB):
                    eng = nc.sync if i % 2 == 0 else nc.scalar
                    eng.dma_start_transpose(
                        out=catT[:, i, :],
                        in_=comb[:, i, :])

                h_ps = psB.tile([128, TPB, 128], f32, tag="h_ps")
                for i in range(TPB):
                    # start only on the first matmul touching each 2KB bank
                    nc.tensor.matmul(
                        h_ps[:, i, :], lhsT=catT[:, i, :], rhs=w1p[:, :],
                        start=(i % 4 == 0), stop=False, skip_group_check=True)
                for q in range(TPB // 4):
                    nc.tensor.matmul(
                        h_ps[:, q * 4:(q + 1) * 4, :],
                        lhsT=onesc[:, :],
                        rhs=b1row[:, q * 512:(q + 1) * 512],
                        start=False, stop=True, skip_group_check=True)
                hr = wp.tile([128, TPB, 128], bf16, tag="hr")
                nc.scalar.activation(hr[:, :, :], h_ps[:, :, :], Act.Relu)
                if P > 0:
                    nc.vector.tensor_reduce(
                        out=gpos[:, c0:c0 + TPB], in_=hr[:, :, 0:P],
                        axis=mybir.AxisListType.X, op=Alu.add)
                if P < H:
                    nc.vector.tensor_reduce(
                        out=gneg[:, c0:c0 + TPB], in_=hr[:, :, P:H],
                        axis=mybir.AxisListType.X, op=Alu.add)

            # ---------------- packed phase ----------------
            nc.scalar.activation(costp[:, :], nsq[:, :], Act.Sqrt)
            nc.vector.tensor_tensor(out=tmp1[:, :], in0=gpos[:, :], in1=gneg[:, :],
                                    op=Alu.subtract)
            nc.scalar.activation(tmp2[:, :], tmp1[:, :], Act.Exp,
                                 bias=scal[:, 0:1], scale=-1.0)
            nc.vector.tensor_scalar_add(tmp2[:, :], tmp2[:, :], 1.0)
            nc.vector.reciprocal(tmp1[:, :], tmp2[:, :])            # gate
            nc.vector.tensor_tensor(out=nsq[:, :], in0=costp[:, :], in1=tmp1[:, :],
                                    op=Alu.mult)                    # gated cost
            nc.scalar.activation(exv[:, :], nsq[:, :], Act.Exp,
                                 bias=0.0, scale=scal[:, 1:2])      # exp(-c/T)
            nc.vector.tensor_tensor(out=exv[:, :], in0=exv[:, :], in1=validm[:, :],
                                    op=Alu.mult)
            nc.vector.tensor_tensor_scan(
                out=tmp1[:, :], data0=contm[:, :], data1=exv[:, :],
                initial=0.0, op0=Alu.mult, op1=Alu.add)             # seg prefix
            nc.vector.tensor_tensor(out=tmp2[:, :], in0=tmp1[:, :], in1=endmm[:, :],
                                    op=Alu.mult)                    # totals at ends

            NT = W // 128

            def reverse_free(dst, src):
                for k in range(NT):
                    t_ps = psA.tile([128, 128], f32, tag="catT_ps")
                    nc.tensor.transpose(
                        out=t_ps[:, :],
                        in_=src[:, (NT - 1 - k) * 128:(NT - k) * 128],
                        identity=ident_f[:, :])
                    t_sb = wp.tile([128, 128], f32, tag="t_sb")
                    nc.scalar.copy(t_sb[:, :], t_ps[:, :])
                    t2_ps = psB.tile([128, 128], f32, tag="h_ps")
                    nc.tensor.transpose(out=t2_ps[:, :], in_=t_sb[:, :],
                                        identity=antid_f[:, :])
                    nc.scalar.copy(dst[:, k * 128:(k + 1) * 128], t2_ps[:, :])

            reverse_free(gpos, tmp2)                                # drev
            nc.vector.tensor_tensor_scan(
                out=gneg[:, :], data0=rcontm[:, :], data1=gpos[:, :],
                initial=0.0, op0=Alu.mult, op1=Alu.add)             # bcast (rev)
            reverse_free(tmp2, gneg)                                # totals/slot
            nc.vector.tensor_tensor(out=tmp2[:, :], in0=tmp2[:, :], in1=invalm[:, :],
                                    op=Alu.add)
            nc.vector.reciprocal(tmp1[:, :], tmp2[:, :])
            nc.vector.tensor_tensor(out=gpos[:, :], in0=exv[:, :], in1=tmp1[:, :],
                                    op=Alu.mult)                    # weights

            nc.sync.dma_start(out=wout_d[:, :], in_=gpos[:, :])
            nc.sync.dma_start(out=cout_d[:, :], in_=nsq[:, :])

    nc.compile()
    _split_excess_waits(nc, mybir)
    return nc


def _get_program(P, NB):
    key = (P, NB)
    if key not in _compiled:
        _compiled[key] = _build_program(P, NB)
    return _compiled[key]


def _make_in_maps(np_inputs):
    import ml_dtypes
    bf = ml_dtypes.bfloat16
    emb_s = np.asarray(np_inputs["embeddings_s"], dtype=np.float32)
    emb_a = np.asarray(np_inputs["embeddings_a"], dtype=np.float32)
    ei = np.asarray(np_inputs["edge_index_sa"])
    W1 = np.asarray(np_inputs["W1"], dtype=np.float32)
    b1 = np.asarray(np_inputs["b1"], dtype=np.float32).reshape(-1)
    W2v = np.asarray(np_inputs["W2"], dtype=np.float32).reshape(-1)
    b2 = np.asarray(np_inputs["b2"], dtype=np.float32).reshape(-1)
    logt = float(np.asarray(np_inputs["log_temperature"]))

    s_idx = ei[0].astype(np.int64)
    a_idx = ei[1].astype(np.int64)

    temp = float(np.exp(logt))
    w2abs = np.abs(W2v)
    pos = np.nonzero(W2v >= 0)[0]
    neg = np.nonzero(W2v < 0)[0]
    perm = np.concatenate([pos, neg])
    P = int(pos.shape[0])
    W1p = np.ascontiguousarray((W1 * w2abs[None, :])[:, perm]).astype(np.float32)
    b1p = (b1 * w2abs)[perm].astype(np.float32)
    b1row = np.tile(b1p, TPB).reshape(1, BATCH).astype(np.float32)
    scal = np.zeros((LANES, 2), dtype=np.float32)
    scal[:, 0] = -b2[0]
    scal[:, 1] = -1.0 / temp
    identm = np.eye(128, dtype=np.float32)
    antidm = np.ascontiguousarray(identm[:, ::-1])

    tab_a = np.zeros((NUM_A // 2, 128), dtype=bf)
    ea_b = emb_a.astype(bf)
    tab_a[:, 0:64] = ea_b[0::2]
    tab_a[:, 64:128] = ea_b[1::2]

    in_maps = []
    preps = []
    max_load = 0
    for c in range(NCORES):
        pr = _prep_core(c, s_idx, a_idx)
        preps.append(pr)
        max_load = max(max_load, pr["max_load"])
        es_b = emb_s[c * S_PER_CORE:(c + 1) * S_PER_CORE].astype(bf)
        tab_s = np.zeros((S_PER_CORE, 256), dtype=bf)
        for k in range(4):
            tab_s[:, k * 64:(k + 1) * 64] = es_b
        in_maps.append({
            "tab_s": tab_s,
            "tab_a": tab_a,
            "idx_s": pr["idx_s16"],
            "idx_a": pr["idx_a16"],
            "parm": pr["parm"],
            "cont": pr["cont"],
            "rev_cont": pr["rev_cont"],
            "endm": pr["endm"],
            "validm": pr["validf"],
            "invalm": pr["invalf"],
            "w1p": W1p,
            "b1row": b1row,
            "scal": scal,
            "identm": identm,
            "antidm": antidm,
        })
    NB = -(-max_load // TPB)
    return P, NB, in_maps, preps


def kernel(embeddings_s, embeddings_a, edge_index_sa, W1, b1, W2, b2,
           log_temperature):
    from concourse.bass_utils import run_bass_kernel_spmd

    np_inputs = {
        "embeddings_s": embeddings_s, "embeddings_a": embeddings_a,
        "edge_index_sa": edge_index_sa, "W1": W1, "b1": b1, "W2": W2,
        "b2": b2, "log_temperature": log_temperature,
    }
    P, NB, in_maps, preps = _make_in_maps(np_inputs)
    nc = _get_program(P, NB)
    res = run_bass_kernel_spmd(nc, in_maps, core_ids=list(range(NCORES)))

    weights = np.zeros(E, dtype=np.float32)
    costs = np.zeros(E, dtype=np.float32)
    for c in range(NCORES):
        pr = preps[c]
        out = res.results[c]
        v = pr["valid"]
        ids = pr["orig"][v]
        weights[ids] = np.asarray(out["wout"])[v]
        costs[ids] = np.asarray(out["cout"])[v]
    return (weights, costs)


# revision 5
# speedup vs baseline: 1.9261x; 1.9261x over previous
"""Trainium2 Bass kernel for DifferentiableEdgeWeighting (8-core SPMD).

Sharding: edges partitioned by source node; core c owns sources
[c*6250, (c+1)*6250). Within a core, segments (source nodes) are bin-packed
onto 128 lanes; each lane stores its segments' edges contiguously (runs
padded to multiples of 4) so the scatter-softmax becomes segmented scans
along the free dimension.

dma_gather cost is pure Q7 descriptor generation (~8 ns/idx, independent of
bytes), so the es side gathers ONE 512B descriptor per 4 slots from a
quad-duplicated table (row r = [es_r,es_r,es_r,es_r]); ea stays per-slot
from the pair-packed table (parity-selected on DVE). catT for the MLP is
built by HWDGE xbar dma transposes (SP/Act queues) instead of PE transposes.

Per-edge pipeline (2048-edge batches of 16 PE tiles):
  - norm: DVE sub/mul + free-dim reduce (bf16).
  - MLP: h = cat @ W1' with W1' = W1*diag(|W2|), columns sign-sorted; gate
    logit = sum(relu(h)[:, :P]) - sum(relu(h)[:, P:]) via free-dim reduces.
  - softmax: exp -> segmented prefix-scan -> totals broadcast back through a
    reversed scan (free-dim flips via PE identity/anti-identity transposes).
"""

import sys
import numpy as np

sys.path.insert(0, "/opt/trn_rl_repo")

NUM_S = 50000
NUM_A = 50000
D = 64
H = 128
E = 1_000_000
NCORES = 8
S_PER_CORE = NUM_S // NCORES  # 6250
LANES = 128
W = 1152                      # slots per lane (quad-padded runs; mult of 128)
TILE = 128
TPB = 16                      # tiles per batch
BATCH = TILE * TPB            # 2048
NQB = BATCH // 4              # es quad idxs per batch (512)
NSLOT = LANES * W

_compiled = {}


def _lane_pack(sizes):
    """Greedy LPT bin-packing of segments onto LANES lanes by padded size."""
    import heapq
    lane_of_seg = np.zeros(sizes.shape[0], dtype=np.int32)
    order = np.argsort(-sizes, kind="stable")
    heap = [(0, l) for l in range(LANES)]
    heapq.heapify(heap)
    for s in order:
        c = int(sizes[s])
        if c == 0:
            continue
        load, l = heapq.heappop(heap)
        lane_of_seg[s] = l
        heapq.heappush(heap, (load + c, l))
    return lane_of_seg


def _wrap16(v):
    """dma_gather index layout: index i lives at [i % 16, i // 16],
    replicated across the 8 Q7 cores (partition groups of 16)."""
    w16 = v.reshape(-1, 16).T  # [16, n/16]
    return np.ascontiguousarray(np.tile(w16, (8, 1)).astype(np.int16))


def _prep_core(c, s_idx, a_idx):
    ids = np.nonzero(s_idx // S_PER_CORE == c)[0]
    sl = (s_idx[ids] - c * S_PER_CORE).astype(np.int32)
    av = a_idx[ids].astype(np.int32)

    o = np.argsort(sl, kind="stable")
    ids, sl, av = ids[o], sl[o], av[o]

    counts = np.bincount(sl, minlength=S_PER_CORE)
    pad4 = (counts + 3) // 4 * 4
    lane_of_seg = _lane_pack(pad4)

    # per-lane segment lists (ascending seg id), then slot streams
    lane_key = lane_of_seg[sl]
    seg_start = np.zeros(S_PER_CORE + 1, dtype=np.int64)
    np.cumsum(counts, out=seg_start[1:])

    grp = np.full((LANES, W), -1, dtype=np.int32)
    orig = np.full((LANES, W), -1, dtype=np.int64)
    amat = np.zeros((LANES, W), dtype=np.int32)
    pos = np.zeros(LANES, dtype=np.int64)
    segs_by_lane = [[] for _ in range(LANES)]
    for s in np.nonzero(counts)[0]:
        segs_by_lane[lane_of_seg[s]].append(s)
    for l in range(LANES):
        p = 0
        for s in segs_by_lane[l]:
            cnt = int(counts[s])
            pc = int(pad4[s])
            e0 = seg_start[s]
            if p + pc > W:
                raise RuntimeError(f"lane overflow: {p + pc} > {W}")
            grp[l, p:p + pc] = s
            orig[l, p:p + cnt] = ids[e0:e0 + cnt]
            amat[l, p:p + cnt] = av[e0:e0 + cnt]
            p += pc
        pos[l] = p
    max_load = int(pos.max())

    valid = orig >= 0
    same = np.zeros((LANES, W), dtype=bool)
    same[:, 1:] = grp[:, 1:] == grp[:, :-1]
    cont = same.astype(np.float32)
    is_end = np.ones((LANES, W), dtype=bool)
    is_end[:, :-1] = ~same[:, 1:]
    endm = is_end.astype(np.float32)
    rev_cont = np.ascontiguousarray((1.0 - endm)[:, ::-1])
    validf = valid.astype(np.float32)
    invalf = (~valid).astype(np.float32)

    # slot stream order: slot t = (lane t%128, pos t//128) -> column-major
    a_slots = np.ascontiguousarray(amat.T).reshape(-1)
    qgrid = grp[:, 0::4]                       # [LANES, W//4]
    q_slots = np.ascontiguousarray(qgrid.T).reshape(-1)
    q_slots = np.where(q_slots < 0, 0, q_slots)
    parm = (amat & 1).astype(np.uint8)

    return {
        "orig": orig, "valid": valid, "max_load": max_load,
        "cont": cont, "rev_cont": rev_cont, "endm": endm,
        "validf": validf, "invalf": invalf,
        "idx_s16": _wrap16(q_slots),
        "idx_a16": _wrap16(a_slots >> 1),
        "parm": parm,
    }


def _split_excess_waits(nc, mybir):
    """Several ISA structs (SWDGE DMA, Ldweights, DVE copies, NoOp) carry a
    single sync-wait slot, but Tile can emit 2+ waits on one instruction.
    Move excess waits onto NoOps inserted just before the instruction on the
    same engine queue (the sequencer stalls on each in order)."""
    fixn = [0]

    def fix_block(blk):
        new_list = []
        for inst in blk.instructions:
            si = inst.sync_info
            if si is not None and si.on_wait and len(si.on_wait) > 1:
                for w in list(si.on_wait[:-1]):
                    nop = mybir.InstNoOp(
                        name=f"I-waitfix-{fixn[0]}",
                        ins=[],
                        outs=[],
                    )
                    fixn[0] += 1
                    nop.engine = inst.engine
                    nop.sync_info = mybir.SyncInfo(on_wait=[w], on_update=[])
                    try:
                        nc.register_instruction(nop, overwrite=True)
                    except Exception:
                        pass
                    new_list.append(nop)
                si.on_wait = [si.on_wait[-1]]
            new_list.append(inst)
        blk.instructions[:] = new_list

    def walk(blocks):
        for b in blocks:
            fix_block(b)
            inner = getattr(b, "blocks", None)
            if inner:
                walk(inner)

    for f in nc.m.functions:
        walk(f.blocks)


def _build_program(P, NB):
    from concourse import bass, mybir, library_config
    from concourse.bacc import Bacc
    import concourse.tile as tile
    from concourse import tile_sem_assignment as _tsa
    _tsa.NUM_SWDGE_GLOBAL_SEMS = 1
    _tsa.NUM_HWDGE_SEMS = 1

    f32 = mybir.dt.float32
    bf16 = mybir.dt.bfloat16
    i16 = mybir.dt.int16
    Alu = mybir.AluOpType
    Act = mybir.ActivationFunctionType

    nc = Bacc()

    NWA = LANES * W // 16        # ea idx columns (full grid)
    NWS = LANES * (W // 4) // 16  # es quad idx columns
    tab_s = nc.declare_dram_parameter("tab_s", [S_PER_CORE, 256], bf16, isOutput=False)
    tab_a = nc.declare_dram_parameter("tab_a", [NUM_A // 2, 128], bf16, isOutput=False)
    idx_s_d = nc.declare_dram_parameter("idx_s", [LANES, NWS], i16, isOutput=False)
    idx_a_d = nc.declare_dram_parameter("idx_a", [LANES, NWA], i16, isOutput=False)
    par_d = nc.declare_dram_parameter("parm", [LANES, W], mybir.dt.uint8, isOutput=False)
    cont_d = nc.declare_dram_parameter("cont", [LANES, W], f32, isOutput=False)
    rcont_d = nc.declare_dram_parameter("rev_cont", [LANES, W], f32, isOutput=False)
    endm_d = nc.declare_dram_parameter("endm", [LANES, W], f32, isOutput=False)
    validm_d = nc.declare_dram_parameter("validm", [LANES, W], f32, isOutput=False)
    invalm_d = nc.declare_dram_parameter("invalm", [LANES, W], f32, isOutput=False)
    w1p_d = nc.declare_dram_parameter("w1p", [H, H], f32, isOutput=False)
    b1row_d = nc.declare_dram_parameter("b1row", [1, BATCH], f32, isOutput=False)
    scal_d = nc.declare_dram_parameter("scal", [LANES, 2], f32, isOutput=False)
    ident_d = nc.declare_dram_parameter("identm", [128, 128], f32, isOutput=False)
    antid_d = nc.declare_dram_parameter("antidm", [128, 128], f32, isOutput=False)

    wout_d = nc.declare_dram_parameter("wout", [LANES, W], f32, isOutput=True)
    cout_d = nc.declare_dram_parameter("cout", [LANES, W], f32, isOutput=True)

    with tile.TileContext(nc) as tc:
        with (
            tc.tile_pool(name="persist", bufs=1) as pp,
            tc.tile_pool(name="work", bufs=2) as wp,
            tc.tile_pool(name="psA", bufs=2, space="PSUM") as psA,
            tc.tile_pool(name="psB", bufs=1, space="PSUM") as psB,
        ):
            nc.gpsimd.load_library(library_config.mlp)

            idx_s = pp.tile([LANES, NWS], i16, tag="idx_s")
            idx_a = pp.tile([LANES, NWA], i16, tag="idx_a")
            parm = pp.tile([LANES, W], mybir.dt.uint8, tag="parm")
            contm = pp.tile([LANES, W], f32, tag="contm")
            rcontm = pp.tile([LANES, W], f32, tag="rcontm")
            endmm = pp.tile([LANES, W], f32, tag="endmm")
            validm = pp.tile([LANES, W], f32, tag="validm")
            invalm = pp.tile([LANES, W], f32, tag="invalm")
            nsq = pp.tile([LANES, W], f32, tag="nsq")
            gpos = pp.tile([LANES, W], f32, tag="gpos")
            gneg = pp.tile([LANES, W], f32, tag="gneg")
            costp = pp.tile([LANES, W], f32, tag="costp")
            exv = pp.tile([LANES, W], f32, tag="exv")
            tmp1 = pp.tile([LANES, W], f32, tag="tmp1")
            tmp2 = pp.tile([LANES, W], f32, tag="tmp2")
            w1p = pp.tile([H, H], bf16, tag="w1p")
            b1row = pp.tile([1, BATCH], bf16, tag="b1row")
            onesc = pp.tile([1, H], bf16, tag="onesc")
            scal = pp.tile([LANES, 2], f32, tag="scal")
            ident_f = pp.tile([128, 128], f32, tag="ident_f")
            antid_f = pp.tile([128, 128], f32, tag="antid_f")

            nc.sync.dma_start(out=idx_s[:, :], in_=idx_s_d[:, :])
            nc.sync.dma_start(out=idx_a[:, :], in_=idx_a_d[:, :])
            nc.sync.dma_start(out=parm[:, :], in_=par_d[:, :])
            nc.sync.dma_start(out=contm[:, :], in_=cont_d[:, :])
            nc.sync.dma_start(out=rcontm[:, :], in_=rcont_d[:, :])
            nc.sync.dma_start(out=endmm[:, :], in_=endm_d[:, :])
            nc.sync.dma_start(out=validm[:, :], in_=validm_d[:, :])
            nc.sync.dma_start(out=invalm[:, :], in_=invalm_d[:, :])
            nc.sync.dma_start(out=scal[:, :], in_=scal_d[:, :])
            nc.sync.dma_start(out=ident_f[:, :], in_=ident_d[:, :])
            nc.sync.dma_start(out=antid_f[:, :], in_=antid_d[:, :])
            nc.gpsimd.dma_start(out=w1p[:, :], in_=w1p_d[:, :])       # f32->bf16
            nc.gpsimd.dma_start(out=b1row[:, :], in_=b1row_d[:, :])   # f32->bf16
            nc.vector.memset(onesc[:, :], 1.0)
            # batches only cover the real prefix; zero the tails read by the
            # packed phase
            nc.vector.memset(nsq[:, :], 0.0)
            nc.vector.memset(gpos[:, :], 0.0)
            nc.vector.memset(gneg[:, :], 0.0)

            for b in range(NB):
                c0 = b * TPB
                ia0 = b * (BATCH // 16)   # ea wrapped-idx column base
                is0 = b * (NQB // 16)     # es wrapped-idx column base
                es_q = wp.tile([128, 4, 256], bf16, tag="es_q")
                ea_t = wp.tile([128, TPB, 128], bf16, tag="ea_t")
                nc.gpsimd.dma_gather(
                    out_ap=es_q[:, :, :], in_ap=tab_s[:, :],
                    idxs_ap=idx_s[:, is0:is0 + NQB // 16],
                    num_idxs=NQB, num_idxs_reg=NQB, elem_size=256,
                    single_packet=False)
                nc.gpsimd.dma_gather(
                    out_ap=ea_t[:, :, :], in_ap=tab_a[:, :],
                    idxs_ap=idx_a[:, ia0:ia0 + BATCH // 16],
                    num_idxs=BATCH, num_idxs_reg=BATCH, elem_size=128,
                    single_packet=False)

                es16 = es_q[:, :, :].rearrange(
                    "p a (b c) -> p (a b) c", b=4, c=64)    # [128, 16t, 64f]

                # comb slot-major [128, 16t, 128f]: f 0:64 = es, 64:128 =
                # easel (= par ? ea_odd : ea_even, parity select in place)
                comb = wp.tile([128, TPB, 128], bf16, tag="comb")
                easel = comb[:, :, 64:128]
                par_bc = parm[:, c0:c0 + TPB].to_broadcast([128, TPB, 64])
                nc.vector.tensor_copy(comb[:, :, 0:64], es16)
                nc.vector.tensor_copy(easel, ea_t[:, :, 0:64])
                nc.vector.copy_predicated(easel, par_bc, ea_t[:, :, 64:128])

                diff = wp.tile([128, TPB, 64], bf16, tag="diff")
                nc.vector.tensor_tensor(
                    out=diff[:, :, :], in0=comb[:, :, 0:64], in1=easel,
                    op=Alu.subtract)
                dsq = wp.tile([128, TPB, 64], bf16, tag="dsq")
                nc.vector.tensor_tensor(
                    out=dsq[:, :, :], in0=diff[:, :, :], in1=diff[:, :, :],
                    op=Alu.mult)
                nc.vector.tensor_reduce(
                    out=nsq[:, c0:c0 + TPB], in_=dsq[:, :, :],
                    axis=mybir.AxisListType.X, op=Alu.add)

                # catT via ONE HWDGE xbar transpose of the whole batch:
                # out[f, t, lane] = comb[lane, t, f] (out row index iterates
                # middle dim outermost: row = t*128 + f)
                catT = wp.tile([128, TPB, 128], bf16, tag="catT")
                nc.sync.dma_start_transpose(
                    out=catT[:, :, :],
                    in_=comb[:, :, :])

                h_ps = psB.tile([128, TPB, 128], f32, tag="h_ps")
                for i in range(TPB):
                    # start only on the first matmul touching each 2KB bank
                    nc.tensor.matmul(
                        h_ps[:, i, :], lhsT=catT[:, i, :], rhs=w1p[:, :],
                        start=(i % 4 == 0), stop=False, skip_group_check=True)
                for q in range(TPB // 4):
                    nc.tensor.matmul(
                        h_ps[:, q * 4:(q + 1) * 4, :],
                        lhsT=onesc[:, :],
                        rhs=b1row[:, q * 512:(q + 1) * 512],
                        start=False, stop=True, skip_group_check=True)
                hr = wp.tile([128, TPB, 128], bf16, tag="hr")
                nc.scalar.activation(hr[:, :, :], h_ps[:, :, :], Act.Relu)
                if P > 0:
                    nc.vector.tensor_reduce(
                        out=gpos[:, c0:c0 + TPB], in_=hr[:, :, 0:P],
                        axis=mybir.AxisListType.X, op=Alu.add)
                if P < H:
                    nc.vector.tensor_reduce(
                        out=gneg[:, c0:c0 + TPB], in_=hr[:, :, P:H],
                        axis=mybir.AxisListType.X, op=Alu.add)

            # ---------------- packed phase ----------------
            nc.scalar.activation(costp[:, :], nsq[:, :], Act.Sqrt)
            nc.vector.tensor_tensor(out=tmp1[:, :], in0=gpos[:, :], in1=gneg[:, :],
                                    op=Alu.subtract)
            nc.scalar.activation(tmp2[:, :], tmp1[:, :], Act.Exp,
                                 bias=scal[:, 0:1], scale=-1.0)
            nc.vector.tensor_scalar_add(tmp2[:, :], tmp2[:, :], 1.0)
            nc.vector.reciprocal(tmp1[:, :], tmp2[:, :])            # gate
            nc.vector.tensor_tensor(out=nsq[:, :], in0=costp[:, :], in1=tmp1[:, :],
                                    op=Alu.mult)                    # gated cost
            nc.scalar.activation(exv[:, :], nsq[:, :], Act.Exp,
                                 bias=0.0, scale=scal[:, 1:2])      # exp(-c/T)
            nc.vector.tensor_tensor(out=exv[:, :], in0=exv[:, :], in1=validm[:, :],
                                    op=Alu.mult)
            nc.vector.tensor_tensor_scan(
                out=tmp1[:, :], data0=contm[:, :], data1=exv[:, :],
                initial=0.0, op0=Alu.mult, op1=Alu.add)             # seg prefix
            nc.vector.tensor_tensor(out=tmp2[:, :], in0=tmp1[:, :], in1=endmm[:, :],
                                    op=Alu.mult)                    # totals at ends

            NT = W // 128

            def reverse_free(dst, src):
                for k in range(NT):
                    t_ps = psA.tile([128, 128], f32, tag="catT_ps")
                    nc.tensor.transpose(
                        out=t_ps[:, :],
                        in_=src[:, (NT - 1 - k) * 128:(NT - k) * 128],
                        identity=ident_f[:, :])
                    t_sb = wp.tile([128, 128], f32, tag="t_sb")
                    nc.scalar.copy(t_sb[:, :], t_ps[:, :])
                    t2_ps = psB.tile([128, 128], f32, tag="h_ps")
                    nc.tensor.transpose(out=t2_ps[:, :], in_=t_sb[:, :],
                                        identity=antid_f[:, :])
                    nc.scalar.copy(dst[:, k * 128:(k + 1) * 128], t2_ps[:, :])

            reverse_free(gpos, tmp2)                                # drev
            nc.vector.tensor_tensor_scan(
                out=gneg[:, :], data0=rcontm[:, :], data1=gpos[:, :],
                initial=0.0, op0=Alu.mult, op1=Alu.add)             # bcast (rev)
            reverse_free(tmp2, gneg)                                # totals/slot
            nc.vector.tensor_tensor(out=tmp2[:, :], in0=tmp2[:, :], in1=invalm[:, :],
                                    op=Alu.add)
            nc.vector.reciprocal(tmp1[:, :], tmp2[:, :])
            nc.vector.tensor_tensor(out=gpos[:, :], in0=exv[:, :], in1=tmp1[:, :],
                                    op=Alu.mult)                    # weights

            nc.sync.dma_start(out=wout_d[:, :], in_=gpos[:, :])
            nc.sync.dma_start(out=cout_d[:, :], in_=nsq[:, :])

    nc.compile()
    _split_excess_waits(nc, mybir)
    return nc


def _get_program(P, NB):
    key = (P, NB)
    if key not in _compiled:
        _compiled[key] = _build_program(P, NB)
    return _compiled[key]


def _make_in_maps(np_inputs):
    import ml_dtypes
    bf = ml_dtypes.bfloat16
    emb_s = np.asarray(np_inputs["embeddings_s"], dtype=np.float32)
    emb_a = np.asarray(np_inputs["embeddings_a"], dtype=np.float32)
    ei = np.asarray(np_inputs["edge_index_sa"])
    W1 = np.asarray(np_inputs["W1"], dtype=np.float32)
    b1 = np.asarray(np_inputs["b1"], dtype=np.float32).reshape(-1)
    W2v = np.asarray(np_inputs["W2"], dtype=np.float32).reshape(-1)
    b2 = np.asarray(np_inputs["b2"], dtype=np.float32).reshape(-1)
    logt = float(np.asarray(np_inputs["log_temperature"]))

    s_idx = ei[0].astype(np.int64)
    a_idx = ei[1].astype(np.int64)

    temp = float(np.exp(logt))
    w2abs = np.abs(W2v)
    pos = np.nonzero(W2v >= 0)[0]
    neg = np.nonzero(W2v < 0)[0]
    perm = np.concatenate([pos, neg])
    P = int(pos.shape[0])
    W1p = np.ascontiguousarray((W1 * w2abs[None, :])[:, perm]).astype(np.float32)
    b1p = (b1 * w2abs)[perm].astype(np.float32)
    b1row = np.tile(b1p, TPB).reshape(1, BATCH).astype(np.float32)
    scal = np.zeros((LANES, 2), dtype=np.float32)
    scal[:, 0] = -b2[0]
    scal[:, 1] = -1.0 / temp
    identm = np.eye(128, dtype=np.float32)
    antidm = np.ascontiguousarray(identm[:, ::-1])

    tab_a = np.zeros((NUM_A // 2, 128), dtype=bf)
    ea_b = emb_a.astype(bf)
    tab_a[:, 0:64] = ea_b[0::2]
    tab_a[:, 64:128] = ea_b[1::2]

    in_maps = []
    preps = []
    max_load = 0
    for c in range(NCORES):
        pr = _prep_core(c, s_idx, a_idx)
        preps.append(pr)
        max_load = max(max_load, pr["max_load"])
        es_b = emb_s[c * S_PER_CORE:(c + 1) * S_PER_CORE].astype(bf)
        tab_s = np.zeros((S_PER_CORE, 256), dtype=bf)
        for k in range(4):
            tab_s[:, k * 64:(k + 1) * 64] = es_b
        in_maps.append({
            "tab_s": tab_s,
            "tab_a": tab_a,
            "idx_s": pr["idx_s16"],
            "idx_a": pr["idx_a16"],
            "parm": pr["parm"],
            "cont": pr["cont"],
            "rev_cont": pr["rev_cont"],
            "endm": pr["endm"],
            "validm": pr["validf"],
            "invalm": pr["invalf"],
            "w1p": W1p,
            "b1row": b1row,
            "scal": scal,
            "identm": identm,
            "antidm": antidm,
        })
    NB = -(-max_load // TPB)
    return P, NB, in_maps, preps


def kernel(embeddings_s, embeddings_a, edge_index_sa, W1, b1, W2, b2,
           log_temperature):
    from concourse.bass_utils import run_bass_kernel_spmd

    np_inputs = {
        "embeddings_s": embeddings_s, "embeddings_a": embeddings_a,
        "edge_index_sa": edge_index_sa, "W1": W1, "b1": b1, "W2": W2,
        "b2": b2, "log_temperature": log_temperature,
    }
    P, NB, in_maps, preps = _make_in_maps(np_inputs)
    nc = _get_program(P, NB)
    res = run_bass_kernel_spmd(nc, in_maps, core_ids=list(range(NCORES)))

    weights = np.zeros(E, dtype=np.float32)
    costs = np.zeros(E, dtype=np.float32)
    for c in range(NCORES):
        pr = preps[c]
        out = res.results[c]
        v = pr["valid"]
        ids = pr["orig"][v]
        weights[ids] = np.asarray(out["wout"])[v]
        costs[ids] = np.asarray(out["cout"])[v]
    return (weights, costs)
